# revision 43
# baseline (speedup 1.0000x reference)
"""Trainium2 Bass kernel for nn_ChiralEmbeddingModel — atom-major v3.

Strategy (8 NeuronCores, pure data-parallel over atoms):
 - host folds all static rescales into the weights (inv-normalization into
   g_w1/g_b1; rms_gamma, 1/sqrt(M), w_cross/w_dot and their path norms into
   W0/Wy1/Wy2; per-atom equivariant-RMS 1/rms skipped: LayerNorm cancels it)
 - host pre-transposes activations to feature-major so the device never
   transposes: eqT [128 m, 2 mh, 3 c, N] fp16, invT [128 i, 2 ih, N] fp16
 - all GEMMs run "atom-major": stationary = feature-major activations
   (128-wide atom blocks), moving = weights; outputs land [atoms, k] in PSUM
   so the cross/dot chain, LayerNorm and the final store need no transposes
 - x0|y1|y2 share one moving pass (wall = [W0|Wy1|Wy2], 192 cols)
 - ACT copies x0|y1 to fp16 SBUF in one op per chunk; DVE does the 6 cross
   products and multiplies by y2 straight from PSUM; GPSIMD (Pool) does the
   subtract + chi accumulation (SBUF fp16 only - Pool cannot touch PSUM)
 - LayerNorm: one multi-group bn_stats per tile; Newton rsqrt batched over
   groups of 4 tiles (kills the serial small-op tail on in-order DVE)
 - sigmoid via tanh; gate merge fused: out = (tanh + 1) * ((chi - mu) * rstd/2)
 - fp16 output, host upcasts to fp32
"""
import os
import sys

sys.path.insert(0, '/opt/trn_rl_repo')

import numpy as np

import concourse.bass as bass
import concourse.bacc as bacc
import concourse.mybir as mybir
import concourse.tile as tile
from concourse.bass_utils import run_bass_kernel_spmd

N, INV, M, K, H = 131072, 256, 256, 64, 512
N_CORES = 8
N_CORE = N // N_CORES          # 16384 atoms per core
T = 512                        # atoms per tile
NT = N_CORE // T               # 32 tiles
G = 8                          # tiles per LayerNorm-tail group
LN_EPS = 1e-5
F16 = mybir.dt.float16
F32 = mybir.dt.float32
I32 = mybir.dt.int32
AF = mybir.ActivationFunctionType
ALU = mybir.AluOpType

LAST_RESULT = None  # BassKernelResults of the most recent run (for profiling)
_NC_CACHE = None


def _ap_view(t, offset_elems, dims):
    """Raw AP on tile t's tensor: partition dim kept, custom free dims."""
    return bass.AP(tensor=t.tensor, offset=t.offset + offset_elems,
                   ap=[list(t.ap[0])] + [list(d) for d in dims])


F8 = mybir.dt.float8e4
DR = mybir.MatmulPerfMode.DoubleRow


def build_nc():
    nc = bacc.Bacc("TRN2", target_bir_lowering=False)
    # one packed input tensor: rows 0-5 eq_hi (mh*3+c), 6-11 eq_lo,
    # 12-13 inv_hi (mh), 14-15 inv_lo
    xin = nc.dram_tensor("xin", [128, 16, N_CORE], F8, kind="ExternalInput")
    # wall = [W0|Wy1|Wy2] * 16 split hi/lo: m1 = hi, m2 = lo16/16, m3 = hi/16
    m1 = nc.dram_tensor("m1", [128, 2, 192], F8, kind="ExternalInput")
    m2 = nc.dram_tensor("m2", [128, 2, 192], F8, kind="ExternalInput")
    m3 = nc.dram_tensor("m3", [128, 2, 192], F8, kind="ExternalInput")
    gw1a = nc.dram_tensor("gw1a", [128, 2, H], F8, kind="ExternalInput")
    gw1b = nc.dram_tensor("gw1b", [128, 2, H], F8, kind="ExternalInput")
    gw1c = nc.dram_tensor("gw1c", [128, 2, H], F8, kind="ExternalInput")
    gb1 = nc.dram_tensor("gb1", [128, 4], F32, kind="ExternalInput")
    gw2 = nc.dram_tensor("gw2", [128, 4, K], F16, kind="ExternalInput")
    out = nc.dram_tensor("out", [NT, 128, 4, K], F16, kind="ExternalOutput")

    with tile.TileContext(nc) as tc:
        with (
            tc.tile_pool(name="const", bufs=1) as const,
            tc.tile_pool(name="inp", bufs=4) as inp,
            tc.tile_pool(name="act", bufs=3) as act,
            tc.tile_pool(name="keep", bufs=G + 3) as keep,
            tc.tile_pool(name="grp", bufs=2) as grp,
            tc.tile_pool(name="ps", bufs=1, space="PSUM") as ps,
        ):
            m1_sb = const.tile([128, 2, 192], F8)
            nc.sync.dma_start(out=m1_sb, in_=m1[:, :, :])
            m2_sb = const.tile([128, 2, 192], F8)
            nc.scalar.dma_start(out=m2_sb, in_=m2[:, :, :])
            m3_sb = const.tile([128, 2, 192], F8)
            nc.scalar.dma_start(out=m3_sb, in_=m3[:, :, :])
            gw1a_sb = const.tile([128, 2, H], F8)
            nc.sync.dma_start(out=gw1a_sb, in_=gw1a[:, :, :])
            gw1b_sb = const.tile([128, 2, H], F8)
            nc.scalar.dma_start(out=gw1b_sb, in_=gw1b[:, :, :])
            gw1c_sb = const.tile([128, 2, H], F8)
            nc.scalar.dma_start(out=gw1c_sb, in_=gw1c[:, :, :])
            gb1_sb = const.tile([128, 4], F32)
            nc.scalar.dma_start(out=gb1_sb, in_=gb1[:, :])
            gw2_sb = const.tile([128, 4, K], F16)
            nc.sync.dma_start(out=gw2_sb, in_=gw2[:, :, :])
            ones4 = const.tile([128, 4, K], F16)
            nc.scalar.activation(out=ones4, in_=gw2_sb, func=AF.Copy,
                                 scale=0.0, bias=1.0)

            chiTs, tanhAs, mvG, pend, apply_q = [], [], None, None, []
            for t in range(NT):
                ti = t % G
                if apply_q:
                    emit_apply(nc, act, apply_q.pop(0), out, ones4)
                in_sb = inp.tile([128, 16, T], F8)
                nc.sync.dma_start(out=in_sb, in_=xin[:, :, t * T:(t + 1) * T])
                invh_v = _ap_view(in_sb, 12 * T, [[T, 2], [1, T]])
                invl_v = _ap_view(in_sb, 14 * T, [[T, 2], [1, T]])

                # ---- gate layer 1: silu((inv @ 16*gw1)/16 + gb1), hi/lo fp8
                # DoubleRow matmuls with error compensation (3 terms)
                g1s = act.tile([128, 4, T], F16)
                for hb in range(4):
                    gp = ps.tile([128, T], F32, tag="g", bufs=2)
                    hs = slice(hb * 128, (hb + 1) * 128)
                    nc.tensor.matmul(gp, gw1a_sb[:, :, hs], invh_v,
                                     perf_mode=DR, start=True, stop=False)
                    nc.tensor.matmul(gp, gw1b_sb[:, :, hs], invl_v,
                                     perf_mode=DR, start=False, stop=False)
                    nc.tensor.matmul(gp, gw1c_sb[:, :, hs], invh_v,
                                     perf_mode=DR, start=False, stop=True)
                    nc.scalar.activation(out=g1s[:, hb, :], in_=gp,
                                         func=AF.Silu, scale=1.0 / 16.0,
                                         bias=gb1_sb[:, hb:hb + 1])

                # ---- x0|y1|y2 atom-major GEMMs + cross/dot chain per 2-ab chunk.
                # Emission is phase-ordered (all Palls, then CRs, then PDs,
                # then chis) so the in-order DVE queue never parks on a
                # cross-engine dependency while ready work waits behind it.
                chiT = keep.tile([128, 4, K], F16)
                As, Ps, CRs = [], [], []
                for ch in range(2):
                    A = ps.tile([128, 3, 2, 256], F32, tag="A", bufs=2)
                    for abi in range(2):
                        ab = ch * 2 + abi
                        for c in range(3):
                            o = A[:, c, abi, 0:192]
                            eh = _ap_view(in_sb, c * T + ab * 128,
                                          [[3 * T, 2], [1, 128]])
                            el = _ap_view(in_sb, (6 + c) * T + ab * 128,
                                          [[3 * T, 2], [1, 128]])
                            nc.tensor.matmul(o, eh, m1_sb,
                                             perf_mode=DR, start=True, stop=False)
                            nc.tensor.matmul(o, el, m3_sb,
                                             perf_mode=DR, start=False, stop=False)
                            nc.tensor.matmul(o, eh, m2_sb,
                                             perf_mode=DR, start=False, stop=True)

                    # x0|y1 -> SBUF fp16: c 0,1 on ACT; c 2 on DVE (balance)
                    Ac = act.tile([128, 3, 2, 128], F16)
                    nc.scalar.copy(out=Ac[:, 0:2], in_=A[:, 0:2, :, 0:128])
                    nc.vector.tensor_copy(out=Ac[:, 2], in_=A[:, 2, :, 0:128])

                    # P products: P[2i] / P[2i+1] pairs for cross components
                    # P0=x0_1*y1_2  P1=x0_2*y1_1 | P2=x0_2*y1_0  P3=x0_0*y1_2
                    # P4=x0_0*y1_1  P5=x0_1*y1_0
                    # Ac free strides: c:256, ab:128, k:1 ; y1 at +64
                    P = act.tile([128, 6, 2, K], F16)
                    pall_specs = [
                        (P[:, 0:2], Ac[:, 1:3, :, 0:K],
                         _ap_view(Ac, 2 * 256 + 64, [[-256, 2], [128, 2], [1, K]])),
                        (P[:, 2:4], _ap_view(Ac, 2 * 256, [[-512, 2], [128, 2], [1, K]]),
                         _ap_view(Ac, 64, [[512, 2], [128, 2], [1, K]])),
                        (P[:, 4:6], Ac[:, 0:2, :, 0:K],
                         _ap_view(Ac, 256 + 64, [[-256, 2], [128, 2], [1, K]])),
                    ]
                    for pi, (o, i0, i1) in enumerate(pall_specs):
                        # one of six product ops per tile runs on Pool (both
                        # inputs are SBUF fp16) to balance DVE vs Pool
                        eng = nc.gpsimd if (ch == 0 and pi == 2) else nc.vector
                        eng.tensor_tensor(out=o, in0=i0, in1=i1, op=ALU.mult)
                    As.append(A)
                    Ps.append(P)

                for ch in range(2):
                    # cross = P_even - P_odd  (Pool) ; P strides: pi:128, ab:64
                    P = Ps[ch]
                    CR = act.tile([128, 3, 2, K], F16)
                    nc.gpsimd.tensor_tensor(
                        out=CR, in0=_ap_view(P, 0, [[256, 3], [K, 2], [1, K]]),
                        in1=_ap_view(P, 128, [[256, 3], [K, 2], [1, K]]),
                        op=ALU.subtract)
                    CRs.append(CR)

                PDs = []
                for ch in range(2):
                    # pd = (cross / 256) * y2: the 16x weight prescale of
                    # x0,y1,y2 cancels here via the fused scalar stage
                    # (y2 straight from PSUM, one PSUM input)
                    PD = act.tile([128, 3, 2, K], F16)
                    nc.vector.scalar_tensor_tensor(
                        out=PD, in0=CRs[ch], scalar=1.0 / 256.0,
                        in1=As[ch][:, :, :, 128:192],
                        op0=ALU.mult, op1=ALU.mult)
                    PDs.append(PD)

                for ch in range(2):
                    # chi = pd_0 + pd_1 + pd_2  (Pool)
                    PD = PDs[ch]
                    cs = chiT[:, ch * 2:(ch + 1) * 2, :]
                    nc.gpsimd.tensor_tensor(out=cs, in0=PD[:, 0], in1=PD[:, 1], op=ALU.add)
                    nc.gpsimd.tensor_tensor(out=cs, in0=cs, in1=PD[:, 2], op=ALU.add)

                # ---- gate layer 2 + tanh (sigmoid = 0.5 + 0.5*tanh(z/2))
                g2p = ps.tile([128, 4, K], F32, tag="g", bufs=2,
                              padded_shape=[128, 4, 128])
                for ab in range(4):
                    for hh in range(4):
                        nc.tensor.matmul(g2p[:, ab, :],
                                         g1s[:, hh, ab * 128:(ab + 1) * 128],
                                         gw2_sb[:, hh, :],
                                         start=(hh == 0), stop=(hh == 3))
                tanhA = keep.tile([128, 4, K], F16)
                nc.scalar.activation(out=tanhA, in_=g2p, func=AF.Tanh, scale=0.5)

                # ---- LayerNorm stats (one multi-group bn_stats, 4 aggrs)
                if ti == 0:
                    mvG = grp.tile([128, G, 4, 2], F32)
                stats6 = act.tile([128, 4, 6], F32)
                for ab in range(4):
                    nc.vector.bn_stats(out=stats6[:, ab, :], in_=chiT[:, ab, :])
                    nc.vector.bn_aggr(out=mvG[:, ti, ab, :], in_=stats6[:, ab, :])
                chiTs.append(chiT)
                tanhAs.append(tanhA)

                # ---- group tail, software-pipelined: the Newton batch is
                # emitted one tile AFTER the group completes (inputs all
                # ready -> no stall on the in-order DVE queue), and the
                # per-tile applies/stores are staggered one per iteration
                if pend is not None:
                    apply_q.extend(emit_group_newton(nc, grp, pend))
                    pend = None
                if ti == G - 1:
                    pend = (mvG, chiTs, tanhAs, t - G + 1)
                    chiTs, tanhAs = [], []
            if pend is not None:
                apply_q.extend(emit_group_newton(nc, grp, pend))
            while apply_q:
                emit_apply(nc, act, apply_q.pop(0), out, ones4)
    nc.compile()
    return nc


def emit_group_newton(nc, grp, pend):
    """Batched Newton rsqrt for one group; returns per-tile apply work items."""
    mvG, chiTs, tanhAs, t0 = pend
    veps = grp.tile([128, G * 4], F32)
    nc.vector.tensor_scalar(
        out=veps, in0=_ap_view(mvG, 1, [[2, G * 4]]),
        scalar1=LN_EPS, scalar2=None, op0=ALU.add)
    ii = grp.tile([128, G * 4], I32)
    nc.vector.tensor_scalar(out=ii, in0=veps.bitcast(I32),
                            scalar1=1, scalar2=-1,
                            op0=ALU.arith_shift_right,
                            op1=ALU.bitwise_xor)
    rstd = grp.tile([128, G * 4], F32)
    nc.vector.tensor_scalar(out=rstd.bitcast(I32), in0=ii,
                            scalar1=0x5f3759df + 1, scalar2=None,
                            op0=ALU.add)
    tN = grp.tile([128, G * 4], F32)
    rstdh = grp.tile([128, G * 4], F32)
    for it in range(2):
        nc.vector.tensor_tensor(out=tN, in0=rstd, in1=rstd, op=ALU.mult)
        nc.vector.tensor_tensor(out=tN, in0=tN, in1=veps, op=ALU.mult)
        nc.vector.tensor_scalar(out=tN, in0=tN, scalar1=-0.5,
                                scalar2=1.5, op0=ALU.mult, op1=ALU.add)
        if it == 0:
            nc.vector.tensor_tensor(out=rstd, in0=rstd, in1=tN, op=ALU.mult)
        else:
            # fold the sigmoid 0.5 into the final rstd
            nc.vector.scalar_tensor_tensor(
                out=rstdh, in0=rstd, scalar=0.5, in1=tN,
                op0=ALU.mult, op1=ALU.mult)

    return [(mvG, rstdh, chiTs[tj], tanhAs[tj], tj, t0 + tj) for tj in range(G)]


def emit_apply(nc, act, item, out, ones4):
    """LN apply + gate merge + store for one tile."""
    mvG, rstdh, chiK, tanhK, tj, oidx = item
    # gfix = tanh + 1 on Pool (TT only there; STT has no DVE fast mode)
    gfix = act.tile([128, 4, K], F16)
    nc.gpsimd.tensor_tensor(out=gfix, in0=tanhK, in1=ones4, op=ALU.add)
    outLN = act.tile([128, 4, K], F16)
    for ab in range(4):
        nc.vector.tensor_scalar(
            out=outLN[:, ab, :], in0=chiK[:, ab, :],
            scalar1=mvG[:, tj, ab, 0:1],
            scalar2=rstdh[:, tj * 4 + ab:tj * 4 + ab + 1],
            op0=ALU.subtract, op1=ALU.mult)
    outF = act.tile([128, 4, K], F16)
    nc.vector.tensor_tensor(out=outF, in0=gfix, in1=outLN, op=ALU.mult)
    # stores go out the ACT queue (HWDGE frees SEQ after descriptor
    # gen): they never park on SP.SEQ blocking the next tile's input
    # DMAs, and unlike SWDGE they don't burn Pool engine time
    nc.scalar.dma_start(out=out[oidx], in_=outF)


try:
    import ml_dtypes
    _F8NP = ml_dtypes.float8_e4m3
except ImportError:  # pragma: no cover
    _F8NP = None


def _hilo8(x):
    """Split x (float) into fp8 hi + fp8 lo where x ~ hi + lo16/16."""
    hi = x.astype(_F8NP)
    lo16 = ((x - hi.astype(np.float64)) * 16.0).astype(_F8NP)
    return hi, lo16


def _prep_weights(mean_inv, std_inv, rms_gamma, W0, W1, W2, w_cross, w_dot,
                  g_w1, g_b1, g_w2):
    g = (rms_gamma.astype(np.float64) / np.sqrt(M))
    W0s = W0.astype(np.float64) * g[:, None]
    Wy1 = (W1.astype(np.float64) * g[:, None]) @ (w_cross.T.astype(np.float64) / np.sqrt(2.0 * K))
    Wy2 = (W2.astype(np.float64) * g[:, None]) @ (w_dot.T.astype(np.float64) / np.sqrt(3.0 * K))
    wall = np.concatenate([W0s, Wy1, Wy2], axis=1)          # [256, 192]
    wall_r = wall.reshape(2, 128, 192).transpose(1, 0, 2) * 16.0
    m1h, m1lo = _hilo8(wall_r)
    m2 = (m1lo.astype(np.float32) / 16.0).astype(_F8NP)
    m3 = (m1h.astype(np.float32) / 16.0).astype(_F8NP)
    inv_std = 1.0 / std_inv.astype(np.float64)
    GW1 = g_w1.astype(np.float64) * inv_std[:, None]
    gw1_r = GW1.reshape(2, 128, H).transpose(1, 0, 2) * 16.0
    g1h, g1lo = _hilo8(gw1_r)
    gw1b = (g1h.astype(np.float32) / 16.0).astype(_F8NP)
    gw1c = (g1lo.astype(np.float32) / 16.0).astype(_F8NP)
    GB1 = g_b1.astype(np.float64) - (mean_inv.astype(np.float64) * inv_std) @ g_w1.astype(np.float64)
    gb1_r = GB1.reshape(4, 128).T.astype(np.float32).copy()
    gw2_r = g_w2.astype(np.float64).reshape(4, 128, K).transpose(1, 0, 2).astype(np.float16)
    return dict(m1=np.ascontiguousarray(m1h), m2=np.ascontiguousarray(m2),
                m3=np.ascontiguousarray(m3),
                gw1a=np.ascontiguousarray(g1h), gw1b=np.ascontiguousarray(gw1b),
                gw1c=np.ascontiguousarray(gw1c),
                gb1=gb1_r, gw2=np.ascontiguousarray(gw2_r))


def kernel(atomic_embeddings, mean_inv, std_inv, rms_gamma, W0, W1, W2,
           w_cross, w_dot, ln_w, ln_b, g_w1, g_b1, g_w2, g_b2):
    global _NC_CACHE, LAST_RESULT
    assert np.allclose(np.asarray(ln_w), 1.0) and np.allclose(np.asarray(ln_b), 0.0), \
        "kernel specialized for ln_w=1, ln_b=0"
    assert np.allclose(np.asarray(g_b2), 0.0), "kernel specialized for g_b2=0"
    weights = _prep_weights(np.asarray(mean_inv), np.asarray(std_inv),
                            np.asarray(rms_gamma), np.asarray(W0), np.asarray(W1),
                            np.asarray(W2), np.asarray(w_cross), np.asarray(w_dot),
                            np.asarray(g_w1), np.asarray(g_b1), np.asarray(g_w2))
    emb = np.asarray(atomic_embeddings)
    if _NC_CACHE is None:
        _NC_CACHE = build_nc()
    nc = _NC_CACHE
    in_maps = []
    for cc in range(N_CORES):
        ec = emb[cc * N_CORE:(cc + 1) * N_CORE]
        inv = ec[:, :INV]
        eq = ec[:, INV:].reshape(N_CORE, M, 3)
        # eqT[p, mh, c, n] = eq[n, mh*128+p, c]
        eqT = np.ascontiguousarray(
            eq.transpose(1, 2, 0).reshape(2, 128, 3, N_CORE).transpose(1, 0, 2, 3))
        invT = np.ascontiguousarray(
            inv.T.reshape(2, 128, N_CORE).transpose(1, 0, 2))
        eh, el = _hilo8(eqT)
        ih, il = _hilo8(invT)
        xin = np.empty((128, 16, N_CORE), dtype=_F8NP)
        xin[:, 0:6] = eh.reshape(128, 6, N_CORE)
        xin[:, 6:12] = el.reshape(128, 6, N_CORE)
        xin[:, 12:14] = ih
        xin[:, 14:16] = il
        m = dict(weights)
        m["xin"] = xin
        in_maps.append(m)
    trace = bool(int(os.environ.get("CHIRAL_TRACE", "0")))
    try:
        from antenv import axon_hooks  # noqa: F401
    except ImportError:
        # NTFF profiling hook absent in this container: tracing would crash
        # inside run_bass_kernel_spmd, so force it off.
        os.environ["BASS_NEVER_TRACE"] = "1"
        trace = False
    res = run_bass_kernel_spmd(nc, in_maps, core_ids=list(range(N_CORES)),
                               trace=trace)
    LAST_RESULT = res
    outs = []
    for cc in range(N_CORES):
        o = res.results[cc]["out"]               # [NT, 128, 4, K] fp16
        outs.append(o.transpose(0, 2, 1, 3).reshape(N_CORE, K).astype(np.float32))
    return np.concatenate(outs, axis=0)
